# revision 13
# baseline (speedup 1.0000x reference)
"""Distributed attention kernel for Trainium2 (8 NeuronCores).

Problem: B=2, L=2048, DIM=1024, H=16 heads, HD=64.
  qkv = x @ Wqkv; q,k = rmsnorm per head (+scales); RoPE(q, k);
  scores = q k^T / sqrt(HD); p = softmax(scores); o = p v;
  out = o @ Wproj + bproj.

Sharding: tensor-parallel over heads -- 2 heads per core (A, B).
Per core, chunked at CH=1024 tokens (4 chunks; 0,1 = batch 0):
  phase 1 (per chunk): qkvT = Wqkv_cols^T @ xT in bf16; rmsnorm inverse
    via exp(-0.5*ln(ssq/64+eps)) on ACT (same table set as softmax exp,
    no table thrash); rope via tc + (P@qn)*st where P is a half-swap
    permutation matmul; v transposed to key-major via PE transposes.
  phase 2 (attention, per batch / l-block of 1024 queries): score
    matmuls for both heads row-packed (K=64 at array rows 0-63 / 64-127,
    concurrent); exp WITHOUT max-subtraction (|s|<=8), split between the
    ACT engine (native exp) and the DVE (Schraudolph bit-trick exp:
    int16(x*128/ln2 + bias) bitcast to bf16, ~2% element rms error) to
    break the ACT throughput ceiling; o-matmul per head with a ones
    column appended to v so the softmax denominator falls out as row 64;
    normalize (recip + ones-broadcast matmul + one DVE mul) before A2A.
  one AllToAll per batch ([1024, 256] bf16, head-dim sharded -> query
    sharded); the b=0 A2A overlaps b=1 attention.
  phase 3: full output projection on the core's 512 queries (+bias).
Host reassembles the per-core query slices.
"""

import sys

if "/opt/trn_rl_repo" not in sys.path:
    sys.path.insert(0, "/opt/trn_rl_repo")

import numpy as np
import ml_dtypes

B, L, DIM, H, HD = 2, 2048, 1024, 16, 64
NC = 8
HPC = H // NC          # heads per core = 2
BL = B * L             # 4096 tokens
CH = 1024              # chunk size (tokens)
NCH = BL // CH         # 4 chunks
EPS = 1e-6
THETA = 10000.0
F = 3 * HPC * HD       # 384 qkv features per core

# fast-exp constants: bits = round(x * 128/ln2 + (128*127 + C))
FE_A = 128.0 / float(np.log(2.0))
FE_C = -7.25
FE_B = 128.0 * 127.0 + FE_C
# which m-iterations route head-B's exp to the DVE fast-exp (of 4)
DVE_EXP = (0, 1, 2)

BF = ml_dtypes.bfloat16
_CACHE = {}


def _rope_tables():
    inv_freq = 1.0 / (THETA ** (np.arange(0, HD, 2, dtype=np.float64) / HD))
    ang = np.arange(L, dtype=np.float64)[None, :] * inv_freq[:, None]  # [32,L]
    return np.cos(ang), np.sin(ang)


def _make_tables(scale, fold):
    """[128, L] f32 cos / (swapped, sign-folded) sin tables, duplicated for
    both heads, per-feature scale and fold factor folded in.

    Device computes, with qn the rms-normalized q and P the half-swap
    permutation (rows 0:32 <-> 32:64 within each 64-row head block):
      out = qn * ct + (P @ qn) * stP
    which equals rotate-half RoPE:
      out[d]    = qn[d]*cos_d*s_d*f    - qn[d+32]*sin_d*s_{d+32}*f   (d<32)
      out[d+32] = qn[d+32]*cos_d*s_{d+32}*f + qn[d]*sin_d*s_d*f
    stP row r multiplies (P@qn)[r] = qn[swap(r)], so stP[r] carries the
    sin coefficient belonging to source row swap(r), sign included.
    """
    c, s = _rope_tables()
    ct = np.empty((HD, L), np.float64)
    st = np.empty((HD, L), np.float64)
    ct[0:32] = c * (scale[0:32, None] * fold)
    ct[32:64] = c * (scale[32:64, None] * fold)
    # out rows 0:32 subtract qn[32:64]*sin*s[32:64]*f -> stP[0:32]
    st[0:32] = -s * (scale[32:64, None] * fold)
    # out rows 32:64 add qn[0:32]*sin*s[0:32]*f -> stP[32:64]
    st[32:64] = s * (scale[0:32, None] * fold)
    ct2 = np.concatenate([ct, ct], axis=0)
    st2 = np.concatenate([st, st], axis=0)
    return ct2.astype(BF), st2.astype(BF)


def _host_inputs(x, Wqkv, q_scale, k_scale, Wproj, bproj):
    x2 = np.ascontiguousarray(np.asarray(x, np.float32).reshape(BL, DIM))
    xT = np.ascontiguousarray(x2.T.astype(BF))              # [DIM, BL]
    Wqkv = np.asarray(Wqkv, np.float32)
    Wq = Wqkv[:, 0 * DIM:1 * DIM].reshape(DIM, H, HD)
    Wk = Wqkv[:, 1 * DIM:2 * DIM].reshape(DIM, H, HD)
    Wv = Wqkv[:, 2 * DIM:3 * DIM].reshape(DIM, H, HD)

    qc, qs = _make_tables(np.asarray(q_scale, np.float64), 1.0 / np.sqrt(HD))
    kc, ks = _make_tables(np.asarray(k_scale, np.float64), 1.0)

    ind2col = np.zeros((128, 2), BF)
    ind2col[0:64, 0] = 1.0
    ind2col[64:128, 1] = 1.0
    indbc = np.zeros((2, 128), BF)
    indbc[0, 0:64] = 1.0
    indbc[1, 64:128] = 1.0
    # P: swap rows 0:32<->32:64 within each 64 block; lhsT = P.T
    P = np.zeros((128, 128), BF)
    for h in range(2):
        r = 64 * h
        for i in range(32):
            P[r + i, r + 32 + i] = 1.0
            P[r + 32 + i, r + i] = 1.0
    permT = np.ascontiguousarray(P.T)
    ones64c = np.ones((1, 64), BF)
    ident = np.eye(128, dtype=BF)
    wp = np.ascontiguousarray(np.asarray(Wproj, np.float32).astype(BF))
    bp = np.ascontiguousarray(
        np.asarray(bproj, np.float32).reshape(8, 128).T)    # [128, 8]

    shared = dict(xT=xT, qc=qc, qs=qs, kc=kc, ks=ks, ind2col=ind2col,
                  indbc=indbc, perm=permT, ones64c=ones64c, ident=ident,
                  wp=wp, bp=bp)
    in_maps = []
    for c in range(NC):
        hA, hB = HPC * c, HPC * c + 1
        wqc = np.concatenate(
            [Wq[:, hA], Wq[:, hB], Wk[:, hA], Wk[:, hB], Wv[:, hA], Wv[:, hB]],
            axis=1)                                        # [DIM, 384]
        m = dict(shared)
        m["wq"] = np.ascontiguousarray(wqc.astype(BF))
        in_maps.append(m)
    return in_maps


def _build(taps=False):
    import concourse.bass as bass  # noqa: F401
    import concourse.mybir as mybir
    import concourse.tile as tile
    from concourse import bacc

    fp32 = mybir.dt.float32
    bf16 = mybir.dt.bfloat16
    i16 = mybir.dt.int16
    AF = mybir.ActivationFunctionType

    nc = bacc.Bacc("TRN2", target_bir_lowering=False, debug=False,
                   num_devices=NC)

    xT = nc.dram_tensor("xT", [DIM, BL], bf16, kind="ExternalInput")
    wq = nc.dram_tensor("wq", [DIM, F], bf16, kind="ExternalInput")
    qc = nc.dram_tensor("qc", [128, L], bf16, kind="ExternalInput")
    qs = nc.dram_tensor("qs", [128, L], bf16, kind="ExternalInput")
    kc = nc.dram_tensor("kc", [128, L], bf16, kind="ExternalInput")
    ks = nc.dram_tensor("ks", [128, L], bf16, kind="ExternalInput")
    ind2col_d = nc.dram_tensor("ind2col", [128, 2], bf16,
                               kind="ExternalInput")
    indbc_d = nc.dram_tensor("indbc", [2, 128], bf16, kind="ExternalInput")
    perm_d = nc.dram_tensor("perm", [128, 128], bf16, kind="ExternalInput")
    ones64c_d = nc.dram_tensor("ones64c", [1, 64], bf16, kind="ExternalInput")
    ident_d = nc.dram_tensor("ident", [128, 128], bf16, kind="ExternalInput")
    wp_d = nc.dram_tensor("wp", [DIM, DIM], bf16, kind="ExternalInput")
    bp_d = nc.dram_tensor("bp", [128, 8], fp32, kind="ExternalInput")
    out_d = nc.dram_tensor("out", [DIM, 512], fp32, kind="ExternalOutput")
    if taps:
        tap_qtn = nc.dram_tensor("tap_qtn", [128, CH], bf16,
                                 kind="ExternalOutput")
        tap_ktn = nc.dram_tensor("tap_ktn", [128, CH], bf16,
                                 kind="ExternalOutput")
        tap_v = nc.dram_tensor("tap_v", [128, 8 * 130], bf16,
                               kind="ExternalOutput")
        tap_pta = nc.dram_tensor("tap_pta", [128, CH], bf16,
                                 kind="ExternalOutput")
        tap_ptb = nc.dram_tensor("tap_ptb", [128, CH], bf16,
                                 kind="ExternalOutput")
        tap_ot = nc.dram_tensor("tap_ot", [65, CH], bf16,
                                kind="ExternalOutput")
        tap_rc = nc.dram_tensor("tap_rc", [1, CH], fp32,
                                kind="ExternalOutput")
        tap_rb = nc.dram_tensor("tap_rb", [64, CH], fp32,
                                kind="ExternalOutput")
        tap_otn = nc.dram_tensor("tap_otn", [64, CH], bf16,
                                 kind="ExternalOutput")
        tap_a2ao = nc.dram_tensor("tap_a2ao", [NC * 128, 128], bf16,
                                  kind="ExternalOutput")

    def mm2(out, lhsT, rhs, start, stop):
        """Matmul with N=1024 rhs split into two N=512 halves (PSUM-bank
        limit: a single matmul's output must fit one 2KB bank)."""
        n = rhs.shape[-1]
        if n <= 512:
            nc.tensor.matmul(out, lhsT, rhs, start=start, stop=stop)
            return
        h = n // 2
        nc.tensor.matmul(out[:, 0:h], lhsT, rhs[:, 0:h],
                         start=start, stop=stop)
        nc.tensor.matmul(out[:, h:n], lhsT, rhs[:, h:n],
                         start=start, stop=stop)

    with tile.TileContext(nc) as tc:
        with (
            tc.tile_pool(name="consts", bufs=1) as consts,
            tc.tile_pool(name="tabs", bufs=1) as tabs,
            tc.tile_pool(name="wqp", bufs=1) as wqp,
            tc.tile_pool(name="qkv_sb", bufs=1) as qkv_sb,
            tc.tile_pool(name="wppool", bufs=1) as wppool,
            tc.tile_pool(name="dram", bufs=1, space="DRAM") as dram,
        ):
            ind2col = consts.tile([128, 2], bf16)
            nc.sync.dma_start(ind2col[:], ind2col_d[:])
            indbc = consts.tile([2, 128], bf16)
            nc.sync.dma_start(indbc[:], indbc_d[:])
            perm = consts.tile([128, 128], bf16)
            nc.sync.dma_start(perm[:], perm_d[:])
            ones64c = consts.tile([1, 64], bf16)
            nc.sync.dma_start(ones64c[:], ones64c_d[:])
            ident = consts.tile([128, 128], bf16)
            nc.sync.dma_start(ident[:], ident_d[:])
            bp_sb = consts.tile([128, 8], fp32)
            nc.sync.dma_start(bp_sb[:], bp_d[:])
            eps_sb = consts.tile([128, 1], fp32)
            nc.gpsimd.memset(eps_sb[:], EPS)

            qc_sb = tabs.tile([128, L], bf16)
            nc.sync.dma_start(qc_sb[:], qc[:])
            qs_sb = tabs.tile([128, L], bf16)
            nc.sync.dma_start(qs_sb[:], qs[:])
            kc_sb = tabs.tile([128, L], bf16)
            nc.sync.dma_start(kc_sb[:], kc[:])
            ks_sb = tabs.tile([128, L], bf16)
            nc.sync.dma_start(ks_sb[:], ks[:])

            wq_sb = []
            for kk in range(8):
                t = wqp.tile([128, F], bf16, name=f"wq{kk}")
                nc.sync.dma_start(t[:], wq[128 * kk:128 * (kk + 1), :])
                wq_sb.append(t)

            qTn = [qkv_sb.tile([128, CH], bf16, name=f"qTn{c}")
                   for c in range(NCH)]
            kTn = [qkv_sb.tile([128, CH], bf16, name=f"kTn{c}")
                   for c in range(NCH)]
            # v key-major: per 128-key block: [64 vA | 1 | 64 vB | 1]
            v_sb = [qkv_sb.tile([128, 8 * 130], bf16, name=f"v{c}")
                    for c in range(NCH)]
            for c in range(NCH):
                nc.gpsimd.memset(v_sb[c][:], 1.0)

            a2a_in = [dram.tile([NC * 128, 128], bf16, name=f"a2a_in{u}")
                      for u in range(4)]
            a2a_out = [dram.tile([NC * 128, 128], bf16, name=f"a2a_out{u}")
                       for u in range(4)]

            # ---------- phase 1: qkv + rmsnorm + rope ----------
            with (
                tc.tile_pool(name="xt", bufs=10) as xtp,
                tc.tile_pool(name="ps", bufs=2, space="PSUM") as ps,
                tc.tile_pool(name="s2", bufs=1, space="PSUM") as s2,
                tc.tile_pool(name="tr", bufs=1, space="PSUM") as tr,
                tc.tile_pool(name="sqp", bufs=2) as sqp,
                tc.tile_pool(name="ivp", bufs=2) as ivp,
                tc.tile_pool(name="qnp", bufs=2) as qnp,
                tc.tile_pool(name="tcp", bufs=4) as tcp,
                tc.tile_pool(name="vtp", bufs=2) as vtp,
            ):
                for ch in range(NCH):
                    c0 = CH * ch
                    lsl = slice(CH * (ch % 2), CH * (ch % 2) + CH)
                    xt = []
                    for kk in range(8):
                        t = xtp.tile([128, CH], bf16, tag="xt")
                        nc.sync.dma_start(
                            t[:], xT[128 * kk:128 * (kk + 1), c0:c0 + CH])
                        xt.append(t)
                    # q and k first (v later so ps pool rotates cleanly)
                    pst = []
                    for tix in range(2):
                        p = ps.tile([128, CH], fp32, tag="ps")
                        for kk in range(8):
                            mm2(p[:], wq_sb[kk][:, 128 * tix:128 * (tix + 1)],
                                xt[kk][:], start=(kk == 0), stop=(kk == 7))
                        pst.append(p)

                    # batched rsqrt: both heads' ssq -> Ln,Ln -> Exp,Exp
                    # (one ACT table-set switch per group, not per op)
                    sq_q = sqp.tile([128, CH], bf16, tag="sq")
                    nc.scalar.activation(sq_q[:], pst[0][:], AF.Square)
                    sq_k = sqp.tile([128, CH], bf16, tag="sq")
                    nc.scalar.activation(sq_k[:], pst[1][:], AF.Square)
                    ssq4 = s2.tile([34, CH], fp32, tag="ssq")
                    mm2(ssq4[0:2, :], ind2col[:], sq_q[:], True, True)
                    mm2(ssq4[32:34, :], ind2col[:], sq_k[:], True, True)
                    lns_q = ivp.tile([2, CH], fp32, tag="lns")
                    nc.scalar.activation(lns_q[:], ssq4[0:2, :], AF.Ln,
                                         bias=eps_sb[0:2, 0:1],
                                         scale=1.0 / HD)
                    lns_k = ivp.tile([2, CH], fp32, tag="lns")
                    nc.scalar.activation(lns_k[:], ssq4[32:34, :], AF.Ln,
                                         bias=eps_sb[0:2, 0:1],
                                         scale=1.0 / HD)
                    ivs_q = ivp.tile([2, CH], bf16, tag="ivs")
                    nc.scalar.activation(ivs_q[:], lns_q[:], AF.Exp,
                                         scale=-0.5)
                    ivs_k = ivp.tile([2, CH], bf16, tag="ivs")
                    nc.scalar.activation(ivs_k[:], lns_k[:], AF.Exp,
                                         scale=-0.5)
                    for tix, (ct, stb, ivs, dst) in enumerate(
                            [(qc_sb, qs_sb, ivs_q, qTn[ch]),
                             (kc_sb, ks_sb, ivs_k, kTn[ch])]):
                        src = pst[tix]
                        invb = tr.tile([128, CH], fp32, tag="tr")
                        mm2(invb[:], indbc[:], ivs[:], True, True)
                        invbs = ivp.tile([128, CH], bf16, tag="invbs")
                        nc.scalar.activation(invbs[:], invb[:], AF.Copy)
                        qn = qnp.tile([128, CH], bf16, tag="qn")
                        nc.vector.tensor_mul(qn[:], src[:], invbs[:])
                        psw = tr.tile([128, CH], fp32, tag="tr")
                        mm2(psw[:], perm[:], qn[:], True, True)
                        tcv = tcp.tile([128, CH], bf16, tag="tc")
                        nc.vector.tensor_mul(tcv[:], qn[:], ct[:, lsl])
                        tsv = tcp.tile([128, CH], bf16, tag="ts")
                        nc.vector.tensor_mul(tsv[:], psw[:], stb[:, lsl])
                        nc.vector.tensor_add(dst[:, :], tcv[:], tsv[:])

                    # v: qkv matmul then transpose to key-major
                    pv = ps.tile([128, CH], fp32, tag="ps")
                    for kk in range(8):
                        mm2(pv[:], wq_sb[kk][:, 256:384],
                            xt[kk][:], start=(kk == 0), stop=(kk == 7))
                    vt = vtp.tile([128, CH], bf16, tag="vt")
                    nc.scalar.activation(vt[:], pv[:], AF.Copy)
                    tp = tr.tile([128, CH], bf16, tag="tr")
                    for blk in range(8):
                        nc.tensor.transpose(
                            tp[:, 128 * blk:128 * (blk + 1)],
                            vt[:, 128 * blk:128 * (blk + 1)], ident[:])
                    for blk in range(8):
                        nc.vector.tensor_copy(
                            v_sb[ch][:, 130 * blk:130 * blk + 64],
                            tp[:, 128 * blk:128 * blk + 64])
                        nc.vector.tensor_copy(
                            v_sb[ch][:, 130 * blk + 65:130 * blk + 129],
                            tp[:, 128 * blk + 64:128 * (blk + 1)])

            # ---------- wproj load (overlaps attention) ----------
            wp_sb = []
            for ff in range(8):
                t = wppool.tile([128, DIM], bf16, name=f"wp{ff}")
                nc.sync.dma_start(t[:], wp_d[128 * ff:128 * (ff + 1), :])
                wp_sb.append(t)

            # ---------- phase 2: attention ----------
            with (
                tc.tile_pool(name="st", bufs=2, space="PSUM") as stp,
                tc.tile_pool(name="po", bufs=2, space="PSUM") as pop,
                tc.tile_pool(name="ptp", bufs=4) as ptp,
                tc.tile_pool(name="otp", bufs=2) as otp,
                tc.tile_pool(name="rcp", bufs=2) as rcp,
                tc.tile_pool(name="onp", bufs=2) as onp,
            ):
                for b in range(2):
                    for lb in range(2):
                        qch = 2 * b + lb
                        poA = pop.tile([65, CH], fp32, tag="po",
                                       name=f"poA{b}{lb}")
                        poB = pop.tile([65, CH], fp32, tag="po",
                                       name=f"poB{b}{lb}")
                        for m in range(16):
                            mc = 2 * b + m // 8
                            mo = 128 * (m % 8)
                            vo = 130 * (m % 8)
                            stA = stp.tile([128, CH], fp32, tag="st")
                            stB = stp.tile([128, CH], fp32, tag="st")
                            # row-packed pair: head A rows 0-63, B 64-127
                            mm2(stA[:], kTn[mc][0:64, mo:mo + 128],
                                qTn[qch][0:64, :], True, True)
                            mm2(stB[:], kTn[mc][64:128, mo:mo + 128],
                                qTn[qch][64:128, :], True, True)
                            ptA = ptp.tile([128, CH], bf16, tag="pt")
                            nc.scalar.activation(ptA[:], stA[:], AF.Exp)
                            ptB = ptp.tile([128, CH], bf16, tag="pt")
                            if (m % 4) in DVE_EXP:
                                nc.vector.tensor_scalar(
                                    ptB[:].bitcast(i16), stB[:],
                                    FE_A, FE_B,
                                    mybir.AluOpType.mult,
                                    mybir.AluOpType.add)
                            else:
                                nc.scalar.activation(ptB[:], stB[:], AF.Exp)
                            if taps and b == 0 and lb == 0 and m == 0:
                                nc.sync.dma_start(tap_pta[:], ptA[:])
                                nc.sync.dma_start(tap_ptb[:], ptB[:])
                            mm2(poA[:], v_sb[mc][:, vo:vo + 65], ptA[:],
                                start=(m == 0), stop=(m == 15))
                            mm2(poB[:], v_sb[mc][:, vo + 65:vo + 130],
                                ptB[:], start=(m == 0), stop=(m == 15))
                        # drain, normalize, stage for A2A
                        for h, poX in ((0, poA), (1, poB)):
                            ot = otp.tile([65, CH], bf16, tag="ot")
                            nc.scalar.activation(ot[:], poX[:], AF.Copy)
                            dn = rcp.tile([1, CH], fp32, tag="dn")
                            nc.scalar.activation(dn[:], poX[64:65, :],
                                                 AF.Copy)
                            rc = rcp.tile([1, CH], fp32, tag="rc")
                            nc.vector.reciprocal_approx_fast(rc[:], dn[:])
                            rcb = rcp.tile([1, CH], bf16, tag="rcb")
                            nc.vector.tensor_copy(rcb[:], rc[:])
                            rb = pop.tile([64, CH], fp32, tag="po")
                            mm2(rb[:], ones64c[:], rcb[:], True, True)
                            otn = onp.tile([64, CH], bf16, tag="otn")
                            nc.vector.tensor_mul(otn[:], ot[0:64, :], rb[:])
                            if taps and b == 0 and lb == 0 and h == 0:
                                nc.sync.dma_start(tap_ot[:], ot[:])
                                nc.sync.dma_start(tap_rc[:], rc[:])
                                nc.sync.dma_start(tap_rb[:], rb[:])
                                nc.sync.dma_start(tap_otn[:], otn[:])
                            for j in range(8):
                                nc.sync.dma_start(
                                    a2a_in[2 * b + lb][
                                        128 * j + 64 * h:
                                        128 * j + 64 * h + 64, :],
                                    otn[:, 128 * j:128 * j + 128])
                        nc.gpsimd.collective_compute(
                            "AllToAll", mybir.AluOpType.bypass,
                            replica_groups=[list(range(NC))],
                            ins=[a2a_in[2 * b + lb][:]],
                            outs=[a2a_out[2 * b + lb][:]],
                        )

            if taps:
                nc.sync.dma_start(tap_qtn[:], qTn[0][:])
                nc.sync.dma_start(tap_ktn[:], kTn[0][:])
                nc.sync.dma_start(tap_v[:], v_sb[0][:])
                nc.sync.dma_start(tap_a2ao[:], a2a_out[0][:])

            # ---------- phase 3: output projection ----------
            with (
                tc.tile_pool(name="ofp", bufs=8) as ofp,
                tc.tile_pool(name="prp", bufs=2, space="PSUM") as prp,
                tc.tile_pool(name="obp", bufs=2) as obp,
            ):
                of = []
                for i in range(8):
                    t = ofp.tile([128, 512], bf16, tag="of", name=f"of{i}")
                    for u in range(4):
                        nc.sync.dma_start(
                            t[:, 128 * u:128 * (u + 1)],
                            a2a_out[u][128 * i:128 * (i + 1), :])
                    of.append(t)
                for d in range(8):
                    pr = prp.tile([128, 512], fp32, tag="pr")
                    for i in range(8):
                        nc.tensor.matmul(
                            pr[:], wp_sb[i][:, 128 * d:128 * (d + 1)],
                            of[i][:], start=(i == 0), stop=(i == 7))
                    ob = obp.tile([128, 512], fp32, tag="ob")
                    nc.scalar.activation(ob[:], pr[:], AF.Identity,
                                         bias=bp_sb[:, d:d + 1])
                    nc.sync.dma_start(out_d[128 * d:128 * (d + 1), :], ob[:])

    nc.compile()
    return nc


def _run(inputs, trace=False, trace_kwargs=None, taps=False):
    from concourse.bass_utils import run_bass_kernel_spmd

    key = "nc_taps" if taps else "nc"
    if key not in _CACHE:
        _CACHE[key] = _build(taps=taps)
    nc = _CACHE[key]
    in_maps = _host_inputs(**inputs)
    res = run_bass_kernel_spmd(
        nc, in_maps, core_ids=list(range(NC)), trace=trace,
        **(trace_kwargs or {}))
    return res


def _assemble(res):
    outT = np.empty((DIM, BL), np.float32)
    for c in range(NC):
        o = res.results[c]["out"]
        for u in range(4):
            qs = 1024 * u + 128 * c
            outT[:, qs:qs + 128] = o[:, 128 * u:128 * (u + 1)]
    return np.ascontiguousarray(outT.T).reshape(B, L, DIM).astype(np.float32)


def kernel(x, Wqkv, q_scale, k_scale, Wproj, bproj):
    res = _run(dict(x=x, Wqkv=Wqkv, q_scale=q_scale, k_scale=k_scale,
                    Wproj=Wproj, bproj=bproj))
    return _assemble(res)


if __name__ == "__main__":
    rng = np.random.default_rng(0)
    x = rng.standard_normal((B, L, DIM), dtype=np.float32)
    Wqkv_ = rng.standard_normal((DIM, 3 * DIM), dtype=np.float32) * DIM ** -0.5
    Wproj_ = rng.standard_normal((DIM, DIM), dtype=np.float32) * DIM ** -0.5
    out = kernel(x=x, Wqkv=Wqkv_, q_scale=np.ones(HD, np.float32),
                 k_scale=np.ones(HD, np.float32), Wproj=Wproj_,
                 bproj=np.zeros(DIM, np.float32))
    print(out.shape, out.dtype)
